# revision 1
# baseline (speedup 1.0000x reference)
"""CQAttention Trainium2 Bass kernel.

Math (per batch, all fp32):
  Ct = C^T (Lc,D); Qt = Q^T (Lq,D); w = [w1,w2,w3]
  S[c,q] = (Ct@w1)[c] + (Qt@w2)[q] + sum_d Ct[c,d]*w3[d]*Qt[q,d]
  S1 = softmax_q(S); S2 = softmax_c(S)
  A = S1@Qt; Bv = (S1@S2^T)@Ct
  out = concat([Ct, A, Ct*A, Ct*Bv], -1)^T   -> (4D, Lc)

Kernel strategy (per core; data-parallel over batch, 4 batches/core):
  * Work in the transposed layout T = S^T (q partitions, c free):
      T = (w3 (.) Q)^T @ C + b[q] + a[c],  a = C^T w1, b = Q^T w2
  * Values |S| <~ 6 so softmax needs no max subtraction:
      E' = exp(T - a) = exp((w3Q)^T C + b)   (exp via ACT with bias=b[q])
    The a[c] factor cancels in softmax_q (S1) entirely; for softmax_c (S2)
    it is folded into the rhs of the S2^T@Ct matmul as exp(a)[k] scaling.
  * Bv reassociated: Bv = S1 @ (S2^T @ Ct) - kills the (Lc,Lc) intermediate.
      M2[q,d] = recip2[q] * sum_k E'[q,k] expa[k] C[d,k]
      r2[q]   = sum_k E'[q,k] expa[k]  (extra rhs column in the same matmul)
      Bv^T    = (M2^T-as-lhsT) @ E' * recip1[c]
      A^T     = (Qt-as-lhsT) @ E' * recip1[c]
      recip1 broadcast over partitions comes from an all-ones 128x128 matmul.
  * a[c] (c on partitions) and b[q] (q on partitions) come from tiny N=1
    matmuls (lhsT = the C/Q blocks, rhs = a w column), batched into one
    PSUM tile so a single ACT exp produces all of expa.
  * Big matmuls and the C/E transposes run as float32r (1 cycle/row vs 4
    for plain fp32); fp32r operands are produced natively by their
    writing instruction (walrus requires a rounding producer).
"""

import functools

import numpy as np

import concourse.bacc as bacc
import concourse.tile as tile
from concourse import mybir
from concourse.bass import ts
from concourse.bass_utils import run_bass_kernel_spmd
from concourse.masks import make_identity

FP = mybir.dt.float32
FPR = mybir.dt.float32r
AF = mybir.ActivationFunctionType

B, D, Lc, Lq = 32, 256, 2048, 256
NCORES = 8
BPC = B // NCORES  # batches per core
DT = D // 128      # 2 d tiles
QT = Lq // 128     # 2 q tiles
KT = Lc // 128     # 16 c(=k) tiles
CH = 512           # matmul rhs chunk (one PSUM bank of fp32)
NJ = Lc // CH      # 4 column chunks


def _body(ctx, tc, C_d, Q_d, w_d, out_d, use_fp32r=True, repeat=1, t_fp32=False):
    nc = tc.nc
    # Matmul-operand tiles use this dtype; their producing instructions
    # perform the fp32 -> fp32r rounding walrus requires.
    MD = FPR if use_fp32r else FP

    singles = ctx.enter_context(tc.tile_pool(name="singles", bufs=1))
    pin = ctx.enter_context(tc.tile_pool(name="pin", bufs=2))
    pbig = ctx.enter_context(tc.tile_pool(name="pbig", bufs=1))
    psm = ctx.enter_context(tc.tile_pool(name="psm", bufs=2))
    pout = ctx.enter_context(tc.tile_pool(name="pout", bufs=2))
    pp_mm = ctx.enter_context(tc.tile_pool(name="pp_mm", bufs=3, space="PSUM"))
    pp_tr = ctx.enter_context(tc.tile_pool(name="pp_tr", bufs=4, space="PSUM"))
    pp_n2 = ctx.enter_context(tc.tile_pool(name="pp_n2", bufs=1, space="PSUM"))

    # --- prefetch first batch inputs so the big loads lead the DMA queue ---
    Cs_pre = pin.tile([128, DT, Lc], FP, tag="Cs", name="Cs_pre")
    Qs_pre = pin.tile([128, DT, Lq], FP, tag="Qs", name="Qs_pre")
    for t in range(DT):
        nc.sync.dma_start(out=Cs_pre[:, t, :], in_=C_d[0, ts(t, 128), :])
        nc.sync.dma_start(out=Qs_pre[:, t, :], in_=Q_d[0, ts(t, 128), :])

    # --- constants ---------------------------------------------------------
    ident = singles.tile([128, 128], FP, tag="ident")
    make_identity(nc, ident)
    identr = singles.tile([128, 128], MD, tag="identr")
    nc.vector.tensor_copy(identr, ident)
    # w1/w2/w3 as per-partition columns, one column per 128-row half of d
    w1c = singles.tile([128, DT], FP, tag="w1c")
    w2c = singles.tile([128, DT], FP, tag="w2c")
    w3c = singles.tile([128, DT], FP, tag="w3c")
    for t in range(DT):
        nc.sync.dma_start(
            out=w1c[:, t : t + 1],
            in_=w_d[ts(t, 128)].rearrange("(p o) -> p o", o=1),
        )
        nc.sync.dma_start(
            out=w2c[:, t : t + 1],
            in_=w_d[D + t * 128 : D + (t + 1) * 128].rearrange("(p o) -> p o", o=1),
        )
        nc.sync.dma_start(
            out=w3c[:, t : t + 1],
            in_=w_d[2 * D + t * 128 : 2 * D + (t + 1) * 128].rearrange(
                "(p o) -> p o", o=1
            ),
        )
    ones_f = singles.tile([128, 128], FP, tag="ones_f")
    nc.vector.memset(ones_f, 1.0)
    ones128 = singles.tile([128, 128], MD, tag="ones")
    nc.vector.tensor_copy(ones128, ones_f)

    # --- per batch ---------------------------------------------------------
    _seq = [b for _ in range(repeat) for b in range(BPC)]
    _pref = {0: (Cs_pre, Qs_pre)}  # tiles whose loads are already emitted
    for _bi, b in enumerate(_seq):
        Cs, Qs = _pref.pop(_bi)

        # rounded copy of C for fp32r matmul streaming (Cs stays exact fp32)
        Csr = pbig.tile([128, DT, Lc], MD, tag="Csr", bufs=2)
        for t in range(DT):
            for j in range(NJ):
                nc.gpsimd.tensor_copy(Csr[:, t, ts(j, CH)], Cs[:, t, ts(j, CH)])

        # prefetch next batch's inputs ahead of this batch's output DMAs
        if _bi + 1 < len(_seq):
            nb = _seq[_bi + 1]
            Cs_n = pin.tile([128, DT, Lc], FP, tag="Cs", name=f"Cs_n{_bi}")
            Qs_n = pin.tile([128, DT, Lq], FP, tag="Qs", name=f"Qs_n{_bi}")
            for t in range(DT):
                nc.sync.dma_start(out=Cs_n[:, t, :], in_=C_d[nb, ts(t, 128), :])
                nc.sync.dma_start(out=Qs_n[:, t, :], in_=Q_d[nb, ts(t, 128), :])
            _pref[_bi + 1] = (Cs_n, Qs_n)

        # wQ = w3 (.) Q (per-partition scale along d)
        wQ = psm.tile([128, DT, Lq], FP if t_fp32 else MD, tag="wQ")
        for t in range(DT):
            nc.vector.tensor_scalar_mul(wQ[:, t, :], Qs[:, t, :], w3c[:, t : t + 1])

        # Qt = Q^T (q parts, d free); b[q] = Q^T w2 via tiny N=1 matmuls
        Qt = psm.tile([128, QT, D], MD, tag="Qt")
        bcol = psm.tile([128, QT], FP, tag="bcol")
        pball = pp_tr.tile([128, QT], FP, tag="ptr", name=f"pball{b}")
        for i in range(QT):
            for j in range(DT):
                p = pp_tr.tile([128, 128], FP, tag="ptr")
                nc.tensor.matmul(
                    p,
                    lhsT=Qs[:, j, ts(i, 128)],
                    rhs=ident,
                    is_transpose=True,
                    start=True,
                    stop=True,
                )
                nc.scalar.activation(Qt[:, i, ts(j, 128)], p, AF.Copy)
                nc.tensor.matmul(
                    pball[:, i : i + 1],
                    lhsT=Qs[:, j, ts(i, 128)],
                    rhs=w2c[:, j : j + 1],
                    start=(j == 0),
                    stop=(j == DT - 1),
                    skip_group_check=True,
                )
        nc.vector.tensor_copy(bcol, pball)

        # T matmul -> E' = exp(T' + b[q])   (q parts, c free)
        E = pbig.tile([128, QT, Lc], MD, tag="E")
        for t in range(QT):
            pT = [pp_mm.tile([128, CH], FP, tag="pmm", name=f"pT{b}_{t}_{j}") for j in range(NJ)]
            for k in range(DT):
                for j in range(NJ):
                    nc.tensor.matmul(
                        pT[j],
                        lhsT=wQ[:, k, ts(t, 128)],
                        rhs=Cs[:, k, ts(j, CH)] if t_fp32 else Csr[:, k, ts(j, CH)],
                        start=(k == 0),
                        stop=(k == DT - 1),
                        skip_group_check=True,
                    )
            for j in range(NJ):
                nc.scalar.activation(
                    E[:, t, ts(j, CH)], pT[j], AF.Exp, bias=bcol[:, t : t + 1]
                )

        # C transpose (fp32r) + a[k] = C^T w1 -> caext = [C^T*expa[k] | expa[k]]
        caext = pbig.tile([128, KT, 258], MD, tag="caext")  # [kp, ki, d|expa|pad]
        expa = psm.tile([128, KT], FP, tag="expa")
        paall = pp_n2.tile([128, KT], FP, tag="pn2", name=f"paall{b}")
        for ki in range(KT):
            for t in range(DT):
                nc.tensor.matmul(
                    paall[:, ki : ki + 1],
                    lhsT=Cs[:, t, ts(ki, 128)],
                    rhs=w1c[:, t : t + 1],
                    start=(t == 0),
                    stop=(t == DT - 1),
                    skip_group_check=True,
                )
        nc.scalar.activation(expa, paall, AF.Exp)
        nc.scalar.activation(caext[:, :, 256:257], paall, AF.Exp)
        nc.scalar.activation(caext[:, :, 257:258], paall, AF.Exp)
        for ki in range(KT):
            for t in range(DT):
                p = pp_tr.tile([128, 128], MD, tag="ptr", name=f"pct{b}_{ki}_{t}")
                nc.tensor.matmul(
                    p,
                    lhsT=Csr[:, t, ts(ki, 128)],
                    rhs=identr,
                    is_transpose=True,
                    start=True,
                    stop=True,
                )
                nc.scalar.activation(
                    caext[:, ki, ts(t, 128)], p, AF.Copy,
                    scale=expa[:, ki : ki + 1],
                )

        # recip1 broadcast to all partitions: all-ones matmul colsum of E'
        r1b = pbig.tile([128, Lc], FP, tag="r1b")
        for j in range(NJ):
            p = pp_mm.tile([128, CH], FP, tag="pmm")
            for t in range(QT):
                nc.tensor.matmul(
                    p,
                    lhsT=ones128,
                    rhs=E[:, t, ts(j, CH)],
                    start=(t == 0),
                    stop=(t == QT - 1),
                )
            nc.vector.reciprocal(r1b[:, ts(j, CH)], p)

        # ET = E'^T (k parts, q free)
        ET = pbig.tile([128, KT, Lq], MD, tag="ET")
        for t in range(QT):
            for ki in range(KT):
                p = pp_tr.tile([128, 128], MD, tag="ptr")
                nc.tensor.matmul(
                    p,
                    lhsT=E[:, t, ts(ki, 128)],
                    rhs=identr,
                    is_transpose=True,
                    start=True,
                    stop=True,
                )
                if ki % 2:
                    nc.scalar.activation(ET[:, ki, ts(t, 128)], p, AF.Copy)
                else:
                    nc.vector.tensor_copy(ET[:, ki, ts(t, 128)], p)

        # N2ext = ET-as-lhsT @ caext : cols 0:256 = unnormalized M2, col 256 = r2
        m2 = psm.tile([128, QT, D], MD, tag="m2")
        rc2 = psm.tile([128, QT], FP, tag="rc2")
        for t in range(QT):
            pn = pp_n2.tile([128, 258], FP, tag="pn2")
            for ki in range(KT):
                nc.tensor.matmul(
                    pn,
                    lhsT=ET[:, ki, ts(t, 128)],
                    rhs=caext[:, ki, :],
                    start=(ki == 0),
                    stop=(ki == KT - 1),
                )
            nc.vector.reciprocal(rc2[:, t : t + 1], pn[:, 256:257])
            nc.vector.tensor_scalar_mul(m2[:, t, :], pn[:, 0:256], rc2[:, t : t + 1])

        # A^T = Qt-as-lhsT @ E' , column-scaled by recip1
        At = pout.tile([128, DT, Lc], FP, tag="At")
        for i in range(DT):
            pA = [pp_mm.tile([128, CH], FP, tag="pmm", name=f"pA{b}_{i}_{j}") for j in range(NJ)]
            for t in range(QT):
                for j in range(NJ):
                    nc.tensor.matmul(
                        pA[j],
                        lhsT=Qt[:, t, ts(i, 128)],
                        rhs=E[:, t, ts(j, CH)],
                        start=(t == 0),
                        stop=(t == QT - 1),
                        skip_group_check=True,
                    )
            for j in range(NJ):
                nc.vector.tensor_mul(At[:, i, ts(j, CH)], pA[j], r1b[:, ts(j, CH)])

        # out rows 0:D = C ; rows D:2D = A^T
        for i in range(DT):
            nc.sync.dma_start(out=out_d[b, ts(i, 128), :], in_=Cs[:, i, :])
            nc.sync.dma_start(out=out_d[b, D + i * 128 : D + (i + 1) * 128, :], in_=At[:, i, :])
        # rows 2D:3D = C (.) A^T (in place after the A^T DMA)
        for i in range(DT):
            nc.gpsimd.tensor_mul(At[:, i, :], At[:, i, :], Cs[:, i, :])
            nc.sync.dma_start(
                out=out_d[b, 2 * D + i * 128 : 2 * D + (i + 1) * 128, :],
                in_=At[:, i, :],
            )

        # Bv^T = M2-as-lhsT @ E' , column-scaled by recip1, then (.) C
        Bt = pout.tile([128, DT, Lc], FP, tag="Bt")
        for i in range(DT):
            pB = [pp_mm.tile([128, CH], FP, tag="pmm", name=f"pB{b}_{i}_{j}") for j in range(NJ)]
            for t in range(QT):
                for j in range(NJ):
                    nc.tensor.matmul(
                        pB[j],
                        lhsT=m2[:, t, ts(i, 128)],
                        rhs=E[:, t, ts(j, CH)],
                        start=(t == 0),
                        stop=(t == QT - 1),
                        skip_group_check=True,
                    )
            for j in range(NJ):
                nc.vector.tensor_mul(Bt[:, i, ts(j, CH)], pB[j], r1b[:, ts(j, CH)])
        for i in range(DT):
            nc.vector.tensor_mul(Bt[:, i, :], Bt[:, i, :], Cs[:, i, :])
            nc.sync.dma_start(
                out=out_d[b, 3 * D + i * 128 : 3 * D + (i + 1) * 128, :],
                in_=Bt[:, i, :],
            )


@functools.lru_cache(maxsize=4)
def build(use_fp32r=True, repeat=1, t_fp32=False):
    import contextlib

    nc = bacc.Bacc("TRN2", target_bir_lowering=False, debug=False)
    C_d = nc.dram_tensor("C", (BPC, D, Lc), FP, kind="ExternalInput").ap()
    Q_d = nc.dram_tensor("Q", (BPC, D, Lq), FP, kind="ExternalInput").ap()
    w_d = nc.dram_tensor("w", (3 * D,), FP, kind="ExternalInput").ap()
    out_d = nc.dram_tensor("out", (BPC, 4 * D, Lc), FP, kind="ExternalOutput").ap()
    with tile.TileContext(nc) as tc:
        with contextlib.ExitStack() as ctx:
            _body(ctx, tc, C_d, Q_d, w_d, out_d, use_fp32r=use_fp32r, repeat=repeat, t_fp32=t_fp32)
    nc.compile()
    return nc


def make_in_maps(C, Q, w):
    C = np.ascontiguousarray(C, dtype=np.float32)
    Q = np.ascontiguousarray(Q, dtype=np.float32)
    w = np.ascontiguousarray(w, dtype=np.float32)
    return [
        {
            "C": C[i * BPC : (i + 1) * BPC],
            "Q": Q[i * BPC : (i + 1) * BPC],
            "w": w,
        }
        for i in range(NCORES)
    ]


def run(C, Q, w, use_fp32r=True, repeat=1, t_fp32=False, **spmd_kwargs):
    nc = build(use_fp32r, repeat, t_fp32)
    res = run_bass_kernel_spmd(
        nc, make_in_maps(C, Q, w), list(range(NCORES)), **spmd_kwargs
    )
    out = np.concatenate([res.results[i]["out"] for i in range(NCORES)], axis=0)
    return out, res


def kernel(C, Q, cmask=None, qmask=None, w=None):
    # cmask/qmask are all-ones for this problem's input spec; with m in {0,1}
    # mask_logits(S, 1) == S, so they do not enter the computation.
    out, _ = run(C, Q, w)
    return out

